# revision 1
# baseline (speedup 1.0000x reference)
"""BetaTCVAE loss kernel for Trainium2 (8 NeuronCores, SPMD).

Math: for z, z_mean, z_logvar in R^[B, L] (B=4096, L=16):
  P_l[i,j] = log N(z[i,l]; mean[j,l], var[j,l])
           = A[i,l]*U[j,l] + B[i,l]*V[j,l] + W[j,l]
    with A = z^2, B = z, U = -0.5*exp(-lv), V = mean*exp(-lv),
         W = -0.5*(mean^2*exp(-lv) + lv + log(2pi))
  log_qz_product[i] = sum_l log sum_j exp(P_l[i,j])
  log_qz[i]         = log sum_j exp(sum_l P_l[i,j])
  out = (w_tc - 1) * mean_i(log_qz - log_qz_product)

Device strategy (shard i across 8 cores, 512 rows each):
  - Rank-3 bilinear structure -> PE builds each [128 i, 512 j] tile of
    P_l with ONE K=12 matmul from fp16 hi/lo splits: contraction rows
    [Hi_w;Lo_w;Hi_w;Lo_w] x [Hi_r;Hi_r;Lo_r;Lo_r] accumulate all four
    hi/lo cross products -> fp32-exact products in PSUM (K is free on PE).
  - The "S" plane (sum_l P_l) is a K=96 matmul pair.
  - ScalarE reads [128, 2048] PSUM spans (4 banks, double-buffered against
    the PE) and applies Exp into bf16 SBUF sinks (mode "dve3", default).
  - VectorE combines each plane's two half-sinks (2x-rate bf16 add) and
    row-sum-reduces once per plane into the acc tile. This beats the
    ScalarE accumulator path (ACTIVATION_READ_ACCUMULATOR costs ~0.5us/..
    ~490ns per instruction on this silicon) and keeps ScalarE at its pure
    1-elem/cycle exp roofline (~257us/core busy).
  - Partial sums [128, 68] DMA out once; host does logs/final mean in f64.

Measured ~266-290us/core steady state (session-dependent); ScalarE is the
bottleneck engine at ~96% occupancy of the kernel span.
"""

import math
import os

# No NTFF hook exists in this container; a stray BASS_TRACE=1 would crash
# run_bass_kernel_spmd on the axon path. Force tracing off.
os.environ["BASS_NEVER_TRACE"] = "1"

import numpy as np
from contextlib import ExitStack

import concourse.bass as bass
import concourse.tile as tile
from concourse import mybir
from concourse.bass_utils import run_bass_kernel_spmd

F32 = mybir.dt.float32
F16 = mybir.dt.float16
BF16 = mybir.dt.bfloat16
EXP = mybir.ActivationFunctionType.Exp

B = 4096
L = 16
N_CORES = 8
I_PER_CORE = B // N_CORES          # 512
N_ITILES = I_PER_CORE // 128       # 4
N_PLANES = L + 1                   # 16 per-dim planes + 1 summed plane
HALF = 2048                        # ACT span (4 PSUM banks)
CHUNK = 512                        # matmul N (1 PSUM bank)
W_TC = 2.0
LOG_2PI = math.log(2.0 * math.pi)

_CACHE = {}


def _split_f16(x):
    hi = x.astype(np.float16)
    lo = (x - hi.astype(np.float64)).astype(np.float16)
    return hi, lo


def _split_multi_waits(nc, keep: int = 1) -> int:
    """This walrus build rejects >1 embedded sem wait per instruction.
    Hoist extras onto standalone same-engine NoOps placed just before."""
    n_split = 0
    for f in nc.m.functions:
        for blk in f.blocks:
            insts = blk.instructions
            if not any(
                i.sync_info is not None and len(i.sync_info.on_wait) > keep
                for i in insts
            ):
                continue
            out = []
            for inst in insts:
                si = inst.sync_info
                if si is not None and len(si.on_wait) > keep:
                    waits = list(si.on_wait)
                    for w in waits[:-keep]:
                        nop = mybir.InstNoOp(
                            name=f"{inst.name}_wsplit{n_split}",
                            ins=[],
                            outs=[],
                            text_hint="split_wait",
                            bass_nofuse=True,
                        )
                        nop.engine = inst.engine
                        nop.sync_info = mybir.SyncInfo(on_wait=[w], on_update=[])
                        out.append(nop)
                        n_split += 1
                    inst.sync_info = mybir.SyncInfo(
                        on_wait=waits[-keep:], on_update=list(si.on_update)
                    )
                out.append(inst)
            blk.instructions = out
    return n_split


def _build_nc(reps: int = 1, mode: str = "dve3", accum_every: int = 8, sink_bufs: int = 3):
    """reps=1: the real kernel. reps>1: same compute wrapped in a hardware
    For_i loop (benchmark mode — device time dominates wall-clock).
    mode="accum": ScalarE accumulator emits row sums.
    mode="dve":   bf16 exp sink + VectorE 4x reduce emits row sums."""
    nc = bass.Bass()
    ltP_d = nc.declare_dram_parameter("ltP", [128, N_ITILES * 512], F16, isOutput=False)
    ltS_d = nc.declare_dram_parameter("ltS", [96, N_ITILES * 128], F16, isOutput=False)
    # rhsP: per q in 0..3 a column block of 4096 (K=12 merged layout)
    rhsP_d = nc.declare_dram_parameter("rhsP", [128, 4 * B], F16, isOutput=False)
    rhsS_d = nc.declare_dram_parameter("rhsS", [96, 2 * B], F16, isOutput=False)
    acc_d = nc.declare_dram_parameter(
        "acc", [128, N_ITILES * N_PLANES * 2], F32, isOutput=True
    )

    with tile.TileContext(nc) as tc, ExitStack() as ctx:
        const = ctx.enter_context(tc.tile_pool(name="const", bufs=1))
        psum = ctx.enter_context(tc.tile_pool(name="psum", bufs=2, space="PSUM"))
        sink_pool = ctx.enter_context(
            tc.tile_pool(name="sink", bufs=1 if mode == "accum" else sink_bufs)
        )

        ltP = const.tile([128, N_ITILES * 512], F16)
        nc.sync.dma_start(ltP[:], ltP_d[:])
        ltS = const.tile([96, N_ITILES * 128], F16)
        nc.sync.dma_start(ltS[:], ltS_d[:])
        rhsP = const.tile([128, 4 * B], F16)
        for q in range(4):
            nc.sync.dma_start(
                rhsP[:, q * B : (q + 1) * B],
                rhsP_d[:, q * B : (q + 1) * B],
            )
        rhsS = const.tile([96, 2 * B], F16)
        nc.sync.dma_start(rhsS[:], rhsS_d[:])

        acc = const.tile([128, N_ITILES * N_PLANES * 2], F32)

        # ACT table warmup: first Exp carries the table load; give it one dep.
        warm = const.tile([128, 1], F32)
        nc.vector.memset(warm[:], 0.0)
        nc.scalar.activation(warm[:], warm[:], EXP)

        def body():
            for t in range(N_ITILES):
                for p in range(N_PLANES):
                    sinks = []
                    for h in range(2):
                        ps = psum.tile([128, HALF], F32, tag="ps")
                        for c in range(4):
                            j0 = h * HALF + c * CHUNK
                            osl = slice(c * CHUNK, (c + 1) * CHUNK)
                            if p < L:
                                g, q = p & 3, p >> 2
                                lt_ap = ltP[32 * g : 32 * g + 12, q * 512 + t * 128 : q * 512 + t * 128 + 128]
                                ra = rhsP[32 * g : 32 * g + 12, q * B + j0 : q * B + j0 + CHUNK]
                                nc.tensor.matmul(
                                    ps[:, osl], lt_ap, ra,
                                    start=True, stop=True, tile_position=(32 * g, 0),
                                )
                            else:
                                lt_ap = ltS[:, t * 128 : (t + 1) * 128]
                                ra = rhsS[:, j0 : j0 + CHUNK]
                                rb = rhsS[:, B + j0 : B + j0 + CHUNK]
                                nc.tensor.matmul(
                                    ps[:, osl], lt_ap, ra,
                                    start=True, stop=False, tile_position=(0, 0),
                                )
                                nc.tensor.matmul(
                                    ps[:, osl], lt_ap, rb,
                                    start=False, stop=True, tile_position=(0, 0),
                                )
                        idx = (t * N_PLANES + p) * 2 + h
                        if mode == "dve3":
                            sink = sink_pool.tile([128, HALF], BF16, tag="sink")
                            nc.scalar.activation(sink[:], ps[:], EXP)
                            sinks.append(sink)
                            if h == 1:
                                nc.vector.tensor_add(
                                    sinks[0][:], sinks[0][:], sinks[1][:]
                                )
                                nc.vector.tensor_reduce(
                                    acc[:, t * N_PLANES + p : t * N_PLANES + p + 1],
                                    sinks[0][:],
                                    axis=mybir.AxisListType.X,
                                    op=mybir.AluOpType.add,
                                )
                            continue
                        use_accum = mode == "accum" or (
                            mode == "hybrid" and idx % accum_every == 0
                        )
                        if use_accum:
                            sink = sink_pool.tile([128, HALF], F32, tag="sinkF")
                            nc.scalar.activation(
                                sink[:], ps[:], EXP, accum_out=acc[:, idx : idx + 1]
                            )
                        else:
                            sink = sink_pool.tile([128, HALF], BF16, tag="sink")
                            nc.scalar.activation(sink[:], ps[:], EXP)
                            nc.vector.tensor_reduce(
                                acc[:, idx : idx + 1], sink[:],
                                axis=mybir.AxisListType.X, op=mybir.AluOpType.add,
                            )

        if reps == 1:
            body()
        else:
            with tc.For_i(0, reps, 1):
                body()

        nc.sync.dma_start(acc_d[:], acc[:])

    _split_multi_waits(nc)
    return nc


def _pack_inputs(z, z_mean, z_logvar):
    """Build per-core input maps (float64 host math, fp16 hi/lo splits)."""
    z = np.asarray(z, np.float64)
    mean = np.asarray(z_mean, np.float64)
    lv = np.asarray(z_logvar, np.float64)

    iv = np.exp(-lv)
    U = -0.5 * iv                                   # [B, L]
    V = mean * iv
    W = -0.5 * (mean * mean * iv + lv + LOG_2PI)
    A = z * z
    Bz = z

    Uh, Ul = _split_f16(U)
    Vh, Vl = _split_f16(V)
    Wh, Wl = _split_f16(W)
    Ah, Al = _split_f16(A)
    Bh, Bl = _split_f16(Bz)

    # rhs tensors are shared across cores
    rhsP = np.zeros((128, 4 * B), np.float16)
    rhsS = np.zeros((96, 2 * B), np.float16)
    for l in range(L):
        g, q = l & 3, l >> 2
        for k, (h_, lo_) in enumerate([(Uh, Ul), (Vh, Vl), (Wh, Wl)]):
            # P planes (K=12 merged): rows [Hi;Hi;Lo;Lo]
            rhsP[32 * g + k, q * B : (q + 1) * B] = h_[:, l]
            rhsP[32 * g + 3 + k, q * B : (q + 1) * B] = h_[:, l]
            rhsP[32 * g + 6 + k, q * B : (q + 1) * B] = lo_[:, l]
            rhsP[32 * g + 9 + k, q * B : (q + 1) * B] = lo_[:, l]
            # S plane: a = [Hi; Lo], b = [Lo; Hi]
            rhsS[3 * l + k, :B] = h_[:, l]
            rhsS[48 + 3 * l + k, :B] = lo_[:, l]
            rhsS[3 * l + k, B:] = lo_[:, l]
            rhsS[48 + 3 * l + k, B:] = h_[:, l]

    ones = np.ones(128, np.float16)
    zer = np.zeros(128, np.float16)
    in_maps = []
    for c in range(N_CORES):
        ltP = np.zeros((128, N_ITILES * 512), np.float16)
        ltS = np.zeros((96, N_ITILES * 128), np.float16)
        for t in range(N_ITILES):
            rows = slice(512 * c + 128 * t, 512 * c + 128 * (t + 1))
            for l in range(L):
                g, q = l & 3, l >> 2
                col = q * 512 + t * 128
                # K=12 merged lhsT: rows [Hi_w; Lo_w; Hi_w; Lo_w]
                for rep in range(2):
                    ltP[32 * g + 6 * rep + 0, col : col + 128] = Ah[rows, l]
                    ltP[32 * g + 6 * rep + 1, col : col + 128] = Bh[rows, l]
                    ltP[32 * g + 6 * rep + 2, col : col + 128] = ones
                    ltP[32 * g + 6 * rep + 3, col : col + 128] = Al[rows, l]
                    ltP[32 * g + 6 * rep + 4, col : col + 128] = Bl[rows, l]
                    ltP[32 * g + 6 * rep + 5, col : col + 128] = zer
                scol = t * 128
                ltS[3 * l + 0, scol : scol + 128] = Ah[rows, l]
                ltS[3 * l + 1, scol : scol + 128] = Bh[rows, l]
                ltS[3 * l + 2, scol : scol + 128] = ones
                ltS[48 + 3 * l + 0, scol : scol + 128] = Al[rows, l]
                ltS[48 + 3 * l + 1, scol : scol + 128] = Bl[rows, l]
                ltS[48 + 3 * l + 2, scol : scol + 128] = zer
        in_maps.append({"ltP": ltP, "ltS": ltS, "rhsP": rhsP, "rhsS": rhsS})
    return in_maps


LAST_RESULT = None


def kernel(z, z_mean, z_logvar):
    global LAST_RESULT
    if "nc" not in _CACHE:
        _CACHE["nc"] = _build_nc()
    nc = _CACHE["nc"]
    in_maps = _pack_inputs(z, z_mean, z_logvar)
    res = run_bass_kernel_spmd(nc, in_maps, list(range(N_CORES)))
    LAST_RESULT = res

    # host reduction in float64 (dve3 layout: one slot per (i-tile, plane))
    diff_sum = 0.0
    for c in range(N_CORES):
        acc = np.asarray(res.results[c]["acc"], np.float64)
        acc = acc[:, : N_ITILES * N_PLANES].reshape(128, N_ITILES, N_PLANES)
        sums = np.transpose(acc, (1, 0, 2)).reshape(I_PER_CORE, N_PLANES)
        log_qz_product = np.sum(np.log(sums[:, :L]), axis=1)
        log_qz = np.log(sums[:, L])
        diff_sum += float(np.sum(log_qz - log_qz_product))
    out = (W_TC - 1.0) * (diff_sum / B)
    return np.float32(out)



# revision 5
# speedup vs baseline: 13.8352x; 13.8352x over previous
"""BetaTCVAE loss kernel for Trainium2 (8 NeuronCores, SPMD).

Math: for z, z_mean, z_logvar in R^[B, L] (B=4096, L=16):
  P_l[i,j] = log N(z[i,l]; mean[j,l], var[j,l])
           = A[i,l]*U[j,l] + B[i,l]*V[j,l] + W[j,l]
    with A = z^2, B = z, U = -0.5*exp(-lv), V = mean*exp(-lv),
         W = -0.5*(mean^2*exp(-lv) + lv + log(2pi))
  log_qz_product[i] = sum_l log sum_j exp(P_l[i,j])
  log_qz[i]         = log sum_j exp(sum_l P_l[i,j])
  out = (w_tc - 1) * mean_i(log_qz - log_qz_product)

Key observation: P_l[i,j] depends on i only through the scalar x = z[i,l],
so  f_l(x) = sum_j exp(P_l(x, j))  is a univariate function (a Gaussian
mixture in x). The 16 per-dim logsumexp planes therefore do NOT need the
full [B, B, L] evaluation: the device tabulates f_l on a G-point uniform
grid covering the z range (G*B*L exps total instead of B^2*L), and the
host interpolates log f_l at the B*L z values with 4-point Lagrange
(final rel err ~ 1e-5, tolerance is 2e-2). Only the summed plane
S = sum_l P_l (log_qz) genuinely needs B^2 work and stays exact.

Device strategy (8 cores):
  - Phase A (S-plane): shard rows i; per core [512 i, 4096 j] via K=96
    fp16 hi/lo matmul pairs (exact products in f32 PSUM), ScalarE Exp into
    bf16 sinks, VectorE tensor_tensor_reduce (add halves + row-sum fused).
  - Phase B (tables): shard j; per core [G grid, 512 j] per-dim planes via
    the K=12 merged hi/lo matmul (one pass), Exp, VectorE per-l row sums.
    Partial tables [G, 16] are summed across cores on the host (tiny).
  - Host (f64, O(B*L)): log of table, Lagrange interpolation, final mean.

ScalarE work/core: (B*4096 + G*512*16)/  = 2.1M + 2.1M (G=256) exps vs
35.7M for the all-on-device baseline (~7x less); ACT and DVE end up
co-bottlenecked at ~30us/core.
"""

import math
import os

# No NTFF hook exists in this container; a stray BASS_TRACE=1 would crash
# run_bass_kernel_spmd on the axon path. Force tracing off.
os.environ["BASS_NEVER_TRACE"] = "1"

import numpy as np
from contextlib import ExitStack

import concourse.bass as bass
import concourse.tile as tile
from concourse import mybir
from concourse.bass_utils import run_bass_kernel_spmd

F32 = mybir.dt.float32
F16 = mybir.dt.float16
BF16 = mybir.dt.bfloat16
EXP = mybir.ActivationFunctionType.Exp

B = 4096
L = 16
N_CORES = 8
I_PER_CORE = B // N_CORES          # 512
N_ITILES = I_PER_CORE // 128       # 4
J_PER_CORE = B // N_CORES          # table j-shard per core
G = 128                            # grid points (multiple of 128)
N_GTILES = G // 128
CHUNK = 512                        # matmul N (1 PSUM bank)
HALF = 2048                        # ACT span (4 PSUM banks)
NACC_A = N_ITILES                  # one S row-sum col per i-tile (fused TTR)
NACC = NACC_A + N_GTILES * 4 * 4   # + per (gtile, q, c) table col
W_TC = 2.0
LOG_2PI = math.log(2.0 * math.pi)

_CACHE = {}


def _split_f16(x):
    hi = x.astype(np.float16)
    lo = (x - hi.astype(np.float64)).astype(np.float16)
    return hi, lo


def _split_multi_waits(nc, keep: int = 1) -> int:
    """This walrus build rejects >1 embedded sem wait per instruction.
    Hoist extras onto standalone same-engine NoOps placed just before."""
    n_split = 0
    for f in nc.m.functions:
        for blk in f.blocks:
            insts = blk.instructions
            if not any(
                i.sync_info is not None and len(i.sync_info.on_wait) > keep
                for i in insts
            ):
                continue
            out = []
            for inst in insts:
                si = inst.sync_info
                if si is not None and len(si.on_wait) > keep:
                    waits = list(si.on_wait)
                    for w in waits[:-keep]:
                        nop = mybir.InstNoOp(
                            name=f"{inst.name}_wsplit{n_split}",
                            ins=[],
                            outs=[],
                            text_hint="split_wait",
                            bass_nofuse=True,
                        )
                        nop.engine = inst.engine
                        nop.sync_info = mybir.SyncInfo(on_wait=[w], on_update=[])
                        out.append(nop)
                        n_split += 1
                    inst.sync_info = mybir.SyncInfo(
                        on_wait=waits[-keep:], on_update=list(si.on_update)
                    )
                out.append(inst)
            blk.instructions = out
    return n_split


def _build_nc(reps: int = 1):
    """reps=1: the real kernel. reps>1: same compute wrapped in a hardware
    For_i loop (benchmark mode - device time dominates wall-clock)."""
    nc = bass.Bass()
    ltSa_d = nc.declare_dram_parameter("ltSa", [96, N_ITILES * 128], F16, isOutput=False)
    ltSb_d = nc.declare_dram_parameter("ltSb", [96, N_ITILES * 128], F16, isOutput=False)
    rhsS_d = nc.declare_dram_parameter("rhsS", [96, B], F16, isOutput=False)
    gridlt_d = nc.declare_dram_parameter("gridlt", [128, 4 * G], F16, isOutput=False)
    gridrhs_d = nc.declare_dram_parameter("gridrhs", [128, 4 * J_PER_CORE], F16, isOutput=False)
    acc_d = nc.declare_dram_parameter("acc", [128, NACC], F32, isOutput=True)

    with tile.TileContext(nc) as tc, ExitStack() as ctx:
        const = ctx.enter_context(tc.tile_pool(name="const", bufs=1))
        psum = ctx.enter_context(tc.tile_pool(name="psum", bufs=2, space="PSUM"))
        sink_pool = ctx.enter_context(tc.tile_pool(name="sink", bufs=4))

        ltSa = const.tile([96, N_ITILES * 128], F16)
        nc.sync.dma_start(ltSa[:], ltSa_d[:])
        ltSb = const.tile([96, N_ITILES * 128], F16)
        nc.sync.dma_start(ltSb[:], ltSb_d[:])
        rhsS = const.tile([96, B], F16)
        nc.sync.dma_start(rhsS[:], rhsS_d[:])
        gridlt = const.tile([128, 4 * G], F16)
        nc.sync.dma_start(gridlt[:], gridlt_d[:])
        gridrhs = const.tile([128, 4 * J_PER_CORE], F16)
        nc.sync.dma_start(gridrhs[:], gridrhs_d[:])

        acc = const.tile([128, NACC], F32)

        # ACT table warmup: first Exp carries the table load; give it one dep.
        warm = const.tile([128, 1], F32)
        nc.vector.memset(warm[:], 0.0)
        nc.scalar.activation(warm[:], warm[:], EXP)

        def body():
            # Phase A: S-plane row sums. Per i-tile t: two j-half spans of
            # [128, 2048], exp'd to bf16 sinks; one fused DVE instr adds the
            # halves and row-sum-reduces into acc[:, t].
            for t in range(N_ITILES):
                sinks = []
                for h in range(2):
                    ps = psum.tile([128, 4, CHUNK], F32, tag="ps")
                    # a,a,a,a then b,b,b,b: lhsT changes once per span
                    for c in range(4):
                        j0 = h * HALF + c * CHUNK
                        nc.tensor.matmul(
                            ps[:, c, :],
                            ltSa[:, t * 128 : (t + 1) * 128],
                            rhsS[:, j0 : j0 + CHUNK],
                            start=True, stop=False, tile_position=(0, 0),
                        )
                    for c in range(4):
                        j0 = h * HALF + c * CHUNK
                        nc.tensor.matmul(
                            ps[:, c, :],
                            ltSb[:, t * 128 : (t + 1) * 128],
                            rhsS[:, j0 : j0 + CHUNK],
                            start=False, stop=True, tile_position=(0, 0),
                        )
                    sink = sink_pool.tile([128, 4, CHUNK], BF16, tag="sink")
                    nc.scalar.activation(sink[:, :, :], ps[:, :, :], EXP)
                    sinks.append(sink)
                nc.vector.tensor_add(
                    sinks[0][:, :, :], sinks[0][:, :, :], sinks[1][:, :, :]
                )
                nc.vector.tensor_reduce(
                    acc[:, t : t + 1],
                    sinks[0][:, :, :],
                    axis=mybir.AxisListType.XY,
                    op=mybir.AluOpType.add,
                )

            # Phase B: per-dim grid tables. Span (gt, q) = grid tile gt,
            # dims l = 4q+c for c in 0..3, over the core's 512-j shard.
            for gt in range(N_GTILES):
                for q in range(4):
                    ps = psum.tile([128, 4, CHUNK], F32, tag="ps")
                    for c in range(4):
                        g = c  # l = 4q + c -> quadrant row band 32c
                        nc.tensor.matmul(
                            ps[:, c, :],
                            gridlt[32 * g : 32 * g + 12, q * G + gt * 128 : q * G + gt * 128 + 128],
                            gridrhs[32 * g : 32 * g + 12, q * CHUNK : (q + 1) * CHUNK],
                            start=True, stop=True, tile_position=(32 * g, 0),
                        )
                    sink = sink_pool.tile([128, 4, CHUNK], BF16, tag="sink")
                    nc.scalar.activation(sink[:, :, :], ps[:, :, :], EXP)
                    col = NACC_A + (gt * 4 + q) * 4
                    nc.vector.tensor_reduce(
                        acc[:, col : col + 4],
                        sink[:, :, :],
                        axis=mybir.AxisListType.X,
                        op=mybir.AluOpType.add,
                    )

        if reps == 1:
            body()
        else:
            with tc.For_i(0, reps, 1):
                body()

        nc.sync.dma_start(acc_d[:], acc[:])

    _split_multi_waits(nc)
    return nc


def _grid_params(z):
    z = np.asarray(z, np.float64)
    lo, hi = float(z.min()), float(z.max())
    h = (hi - lo) / (G - 7)
    g0 = lo - 3.0 * h
    return g0, h


def _pack_inputs(z, z_mean, z_logvar):
    """Build per-core input maps (float64 host math, fp16 hi/lo splits)."""
    z = np.asarray(z, np.float64)
    mean = np.asarray(z_mean, np.float64)
    lv = np.asarray(z_logvar, np.float64)

    iv = np.exp(-lv)
    U = -0.5 * iv                                   # [B, L]
    V = mean * iv
    W = -0.5 * (mean * mean * iv + lv + LOG_2PI)
    A = z * z
    Bz = z

    Uh, Ul = _split_f16(U)
    Vh, Vl = _split_f16(V)
    Wh, Wl = _split_f16(W)
    Ah, Al = _split_f16(A)
    Bh, Bl = _split_f16(Bz)

    g0, h = _grid_params(z)
    grid = g0 + h * np.arange(G)
    Gh, Gl = _split_f16(grid)            # B-coefficient of grid rows
    G2h, G2l = _split_f16(grid * grid)   # A-coefficient

    # gridlt: K=12 merged hi/lo lhsT over grid rows; shared across cores.
    # For l: g=l&3 band rows 32g..32g+11, cols q*G + gt*128 (q=l>>2) - but
    # the [12,128] block is the same for every l in a band/q (it's just the
    # grid), so fill per band/q directly.
    ones = np.ones(G, np.float16)
    zer = np.zeros(G, np.float16)
    gridlt = np.zeros((128, 4 * G), np.float16)
    block = np.stack([G2h, Gh, ones, G2l, Gl, zer] * 2)  # [12, G]
    for g in range(4):
        for q in range(4):
            gridlt[32 * g : 32 * g + 12, q * G : (q + 1) * G] = block

    in_maps = []
    onesB, zerB = np.ones(128, np.float16), np.zeros(128, np.float16)
    for c in range(N_CORES):
        # S-plane lhsT pair: a = [Hi_w; Lo_w], b = [Lo_w; Hi_w]
        ltSa = np.zeros((96, N_ITILES * 128), np.float16)
        for t in range(N_ITILES):
            rows = slice(512 * c + 128 * t, 512 * c + 128 * (t + 1))
            col = slice(t * 128, (t + 1) * 128)
            for l in range(L):
                ltSa[3 * l + 0, col] = Ah[rows, l]
                ltSa[3 * l + 1, col] = Bh[rows, l]
                ltSa[3 * l + 2, col] = onesB
                ltSa[48 + 3 * l + 0, col] = Al[rows, l]
                ltSa[48 + 3 * l + 1, col] = Bl[rows, l]
                ltSa[48 + 3 * l + 2, col] = zerB
        ltSb = np.concatenate([ltSa[48:], ltSa[:48]], axis=0)

        # S-plane rhs: rows [Hi_r(48); Lo_r(48)], all B j's (replicated).
        # gridrhs: per-dim U,V,W hi/lo for the core's j-shard, K=12 layout.
        if c == 0:
            rhsS = np.zeros((96, B), np.float16)
            for l in range(L):
                rhsS[3 * l + 0] = Uh[:, l]
                rhsS[3 * l + 1] = Vh[:, l]
                rhsS[3 * l + 2] = Wh[:, l]
                rhsS[48 + 3 * l + 0] = Ul[:, l]
                rhsS[48 + 3 * l + 1] = Vl[:, l]
                rhsS[48 + 3 * l + 2] = Wl[:, l]

        jsh = slice(J_PER_CORE * c, J_PER_CORE * (c + 1))
        gridrhs = np.zeros((128, 4 * J_PER_CORE), np.float16)
        for l in range(L):
            g, q = l & 3, l >> 2
            cols = slice(q * J_PER_CORE, (q + 1) * J_PER_CORE)
            for k, (h_, lo_) in enumerate([(Uh, Ul), (Vh, Vl), (Wh, Wl)]):
                gridrhs[32 * g + k, cols] = h_[jsh, l]
                gridrhs[32 * g + 3 + k, cols] = h_[jsh, l]
                gridrhs[32 * g + 6 + k, cols] = lo_[jsh, l]
                gridrhs[32 * g + 9 + k, cols] = lo_[jsh, l]

        in_maps.append({
            "ltSa": ltSa, "ltSb": ltSb, "rhsS": rhsS,
            "gridlt": gridlt, "gridrhs": gridrhs,
        })
    return in_maps


LAST_RESULT = None


def kernel(z, z_mean, z_logvar):
    global LAST_RESULT
    if "nc" not in _CACHE:
        _CACHE["nc"] = _build_nc()
    nc = _CACHE["nc"]
    in_maps = _pack_inputs(z, z_mean, z_logvar)
    res = run_bass_kernel_spmd(nc, in_maps, list(range(N_CORES)))
    LAST_RESULT = res

    # Host reduction in float64.
    z64 = np.asarray(z, np.float64)
    g0, h = _grid_params(z64)

    # S-plane: acc[p, t] on core c = sum_j exp(S[i, j]) for i = 512c+128t+p
    sums_S = np.zeros(B)
    ftab = np.zeros((G, L))
    for c in range(N_CORES):
        acc = np.asarray(res.results[c]["acc"], np.float64)
        for t in range(N_ITILES):
            sums_S[512 * c + 128 * t : 512 * c + 128 * (t + 1)] = acc[:, t]
        # (cols 0..3 are full row sums: halves added on-device before reduce)
        # tables: col NACC_A + (gt*4+q)*4 + cc  <->  l = 4q+cc, grid gt*128+p
        tb = acc[:, NACC_A:].reshape(128, N_GTILES, 4, 4)  # [p, gt, q, cc]
        ftab += np.transpose(tb, (1, 0, 2, 3)).reshape(G, L)
    log_qz = np.log(sums_S)

    gtab = np.log(ftab)  # [G, L]
    t = (z64 - g0) / h
    i0 = np.clip(np.floor(t).astype(int), 1, G - 3)
    f = t - i0
    w0 = -f * (f - 1) * (f - 2) / 6
    w1 = (f + 1) * (f - 1) * (f - 2) / 2
    w2 = -(f + 1) * f * (f - 2) / 2
    w3 = (f + 1) * f * (f - 1) / 6
    cols = np.arange(L)[None, :].repeat(B, 0)
    lqp = (w0 * gtab[i0 - 1, cols] + w1 * gtab[i0, cols]
           + w2 * gtab[i0 + 1, cols] + w3 * gtab[i0 + 2, cols]).sum(axis=1)

    out = (W_TC - 1.0) * float(np.mean(log_qz - lqp))
    return np.float32(out)
